# revision 16
# baseline (speedup 1.0000x reference)
"""DenseGGNN (gnn_message_passing) Trainium2 Bass kernel.

Math per layer i (per batch):
    s  = A^T @ h                    # [N, C], A binary adjacency
    gx = s @ (W_i @ w_ih_i^T)       # fused:  ((A^T h) W) @ w_ih^T
    gh = h @ w_hh_i^T
    r  = sigmoid(gx_r + gh_r + b_r);  zc = 1 - z = sigmoid(-(gx_z + gh_z + b_z))
    n  = tanh(gx_n + b_in + r * (gh_n + b_hn))
    h' = h + zc * (n - h)

Device layout ("T-layout"): state hT is feature-major fp16 [C=128 part,
N=1024].  The whole matmul path is single-pass fp16: the adjacency is
exact in fp16 (binary), h/weights are rounded to fp16, PSUM accumulates
in fp32.  Final rel err ~1e-3 against the fp32 reference -- well inside
the 2e-2 gate (the earlier split-fp16 hi/lo scheme reached 3e-4 but at
1.7x the PE work).

x arrives pre-cast to fp16 in BOTH layouts (node-major = layer-0
stationary state, feature-major = layer-0 hT), and y leaves the device
feature-major fp16 (host un-transposes + widens), so the only on-device
layout changes are the three inter-layer xbar transposes per batch.
The hn/xn PSUM banks are drained immediately by DVE tensor_scalar ops
so bank recycling never gates the PE, and per layer the s-matmuls run
one batch ahead of the gate phase so the PE always has independent
work to absorb the gate/elementwise chain latency.

Sharding: batch (32) split across 8 cores, 4 batches/core, weights
replicated; no cross-core communication.
"""

from contextlib import ExitStack, nullcontext

import numpy as np

import concourse.bass as bass
import concourse.bacc as bacc
import concourse.tile as tile
import concourse.mybir as mybir
from concourse.bass_utils import run_bass_kernel_spmd

B, N, C, L = 32, 1024, 128, 4
NCORES = 8
BPC = B // NCORES          # batches per core
P = 128                    # partitions
NT = N // P                # node tiles (8)
HALF = 512                 # psum-bank-sized column chunk
WSPL = 2                   # layers < WSPL use split-fp16 gate weights

F32 = mybir.dt.float32
F16 = mybir.dt.float16
F32R = mybir.dt.float32r
AF = mybir.ActivationFunctionType
ALU = mybir.AluOpType

_PROGRAM_CACHE = {}


def _build_program(reps: int = 1, loop_reps: int = 1) -> bass.Bass:
    # reps > 1 re-emits the whole body back-to-back in one NEFF;
    # loop_reps > 1 wraps the body in a hardware For_i loop.  Both are
    # benchmarking aids (wall-time slope isolates per-iteration device
    # time from the axon dispatch overhead).
    nc = bacc.Bacc()

    x_d = nc.declare_dram_parameter("x", [BPC, N, C], F16, isOutput=False)
    xlo_d = nc.declare_dram_parameter("xlo", [BPC, N, C], F16, isOutput=False)
    xT_d = nc.declare_dram_parameter("xT", [BPC, C, N], F16, isOutput=False)
    adj_d = nc.declare_dram_parameter("adj", [BPC, N, N], F16, isOutput=False)
    wch_d = nc.declare_dram_parameter("wch", [C, L, 3, C], F32R, isOutput=False)
    wch16_d = nc.declare_dram_parameter("wch16", [C, WSPL, 3, C], F16, isOutput=False)
    wcl16_d = nc.declare_dram_parameter("wcl16", [C, WSPL, 3, C], F16, isOutput=False)
    whh_d = nc.declare_dram_parameter("whh", [C, L, 3, C], F16, isOutput=False)
    bias_d = nc.declare_dram_parameter("bias", [C, L, 4], F32, isOutput=False)
    y_d = nc.declare_dram_parameter("y", [BPC, C, N], F16, isOutput=True)

    with tile.TileContext(nc) as tc, ExitStack() as ctx:
        consts = ctx.enter_context(tc.tile_pool(name="consts", bufs=1))
        adj_pool = ctx.enter_context(tc.tile_pool(name="adjp", bufs=1))
        hnm_pool = ctx.enter_context(tc.tile_pool(name="hnm", bufs=1))
        hT_pool = ctx.enter_context(tc.tile_pool(name="hT", bufs=2))
        sT_pool = ctx.enter_context(tc.tile_pool(name="sT", bufs=3))
        ew_pool = ctx.enter_context(tc.tile_pool(name="ew", bufs=14))
        ps_s = ctx.enter_context(tc.tile_pool(name="ps_s", bufs=2, space="PSUM"))
        ps_g = ctx.enter_context(tc.tile_pool(name="ps_g", bufs=6, space="PSUM"))

        def wslice(w, i, g):
            return w[:, (i * 3 + g) * C:(i * 3 + g + 1) * C]

        def bslice(i, k):
            return bias[:, i * 4 + k:i * 4 + k + 1]

        loop_cm = (tc.For_i(0, loop_reps, 1, hint_engines=(mybir.EngineType.PE,))
                   if loop_reps > 1 else nullcontext())
        with loop_cm:
          for _rep in range(reps):
            # ---- input loads -------------------------------------------------
            # Everything rides the SP queue (keeping the ACT/DVE sequencers
            # free for compute), batch-interleaved so batch b's working set
            # lands before batch b+1's.  adj is chunked by column halves so
            # the first s-matmul (which consumes all row tiles of column half
            # 0) can start after 1 MB.
            adj_sb = []
            x_lo = []
            h_nm = [None] * BPC
            hT = [None] * BPC
            wch = consts.tile([P, L * 3 * C], F32R)
            wch16 = consts.tile([P, WSPL * 3 * C], F16)
            wcl16 = consts.tile([P, WSPL * 3 * C], F16)
            whh = consts.tile([P, L * 3 * C], F16)
            bias = consts.tile([P, L * 4], F32)
            for b in range(BPC):
                hi = hnm_pool.tile([P, NT, C], F16, tag=f"hnm{b}")
                nc.sync.dma_start(hi[:], x_d[b].rearrange("(t p) c -> p t c", p=P))
                h_nm[b] = hi
                a = adj_pool.tile([P, NT, N], F16, tag=f"adj{b}")
                src = adj_d[b].rearrange("(t p) n -> p t n", p=P)
                nc.sync.dma_start(a[:, :, 0:HALF], src[:, :, 0:HALF])
                lo = hnm_pool.tile([P, NT, C], F16, tag=f"hlo{b}", name=f"xlo{b}")
                nc.sync.dma_start(lo[:], xlo_d[b].rearrange("(t p) c -> p t c", p=P))
                x_lo.append(lo)
                if b == 0:
                    nc.sync.dma_start(wch16[:],
                                      wch16_d.rearrange("c l g d -> c (l g d)"))
                    nc.sync.dma_start(wcl16[:],
                                      wcl16_d.rearrange("c l g d -> c (l g d)"))
                hh = hT_pool.tile([P, N], F16, tag=f"hT{b}")
                nc.sync.dma_start(hh[:], xT_d[b])
                hT[b] = hh
                nc.sync.dma_start(a[:, :, HALF:], src[:, :, HALF:])
                if b == 0:
                    nc.sync.dma_start(whh[:], whh_d.rearrange("c l g d -> c (l g d)"))
                    nc.sync.dma_start(bias[:], bias_d.rearrange("c l k -> c (l k)"))
                    nc.sync.dma_start(wch[:], wch_d.rearrange("c l g d -> c (l g d)"))
                adj_sb.append(a)

            # ---- layers ------------------------------------------------------
            # Emission order is staged per engine to avoid head-of-line
            # blocking on the in-order sequencer queues:
            #   PE : s(0) s(1) g(0) s(2) g(1) s(3) g(2) g(3)   (1 batch ahead)
            #   ACT: copies(0) [rz(b) copies(b+1) tanh(b)] ...  (tanh last --
            #        its input arrives late; copies must not queue behind it)
            #   DVE: [drains(b) t/u(b) nh(b)] ...
            def emit_s_mm(i, b):
                s = sT_pool.tile([P, N], F16 if i < WSPL else F32R, tag="s")
                stats = [h_nm[b], x_lo[b]] if i == 0 else [h_nm[b]]
                pss = []
                for half in range(2):
                    hs = slice(half * HALF, (half + 1) * HALF)
                    ps = ps_s.tile([P, HALF], F32, tag="ps_s")
                    for ti, hnm in enumerate(stats):
                        for j in range(NT):
                            nc.tensor.matmul(
                                ps[:],
                                lhsT=hnm[:, j, :],
                                rhs=adj_sb[b][:, j, hs],
                                start=(ti == 0 and j == 0),
                                stop=(ti == len(stats) - 1 and j == NT - 1),
                            )
                    pss.append(ps)
                return s, pss

            def emit_s_copies(s_ps):
                s, pss = s_ps
                for half in range(2):
                    hs = slice(half * HALF, (half + 1) * HALF)
                    nc.scalar.activation(s[:, hs], pss[half], AF.Copy)

            def emit_nh(i, b, old_hT, new_h, e):
                # final h' = h + zc*(n - h); deferred one batch stage so the
                # next batch's psum drains never queue behind this chain tail
                last_layer = i == L - 1
                for nh in range(2):
                    sl = slice(nh * HALF, (nh + 1) * HALF)
                    nc.vector.tensor_add(new_h[:, sl], old_hT[:, sl], e[nh][:])
                    if last_layer:
                        # store feature-major fp16; host un-transposes
                        nc.sync.dma_start(out=y_d[b][:, sl], in_=new_h[:, sl])
                if not last_layer:
                    nhi = hnm_pool.tile([P, NT, C], F16, tag=f"hnm{b}")
                    nc.sync.dma_start(out=nhi[:], in_=new_h[:], transpose=True)
                    h_nm[b] = nhi

            def emit_gates(i, b, s_ps, s_copies_next, nh_prev):
                s, _ = s_ps
                new_h = hT_pool.tile([P, N], F16, tag=f"hT{b}")

                banks = []
                for nh in range(2):
                    sl = slice(nh * HALF, (nh + 1) * HALF)
                    pr = ps_g.tile([P, HALF], F32, tag="psg")
                    phn = ps_g.tile([P, HALF], F32, tag="psg")
                    pz = ps_g.tile([P, HALF], F32, tag="psg")
                    pxn = ps_g.tile([P, HALF], F32, tag="psg")
                    def gx_mm(pg, g, is_first, with_gh):
                        # early layers: fp16 hi+lo weight split (the fused
                        # gate weight's fp16 rounding error would amplify
                        # ~4x/layer); late layers: single-pass fp32r
                        if i < WSPL:
                            nc.tensor.matmul(pg[:], lhsT=wslice(wch16, i, g),
                                             rhs=s[:, sl], start=is_first,
                                             stop=False)
                            nc.tensor.matmul(pg[:], lhsT=wslice(wcl16, i, g),
                                             rhs=s[:, sl], start=False,
                                             stop=not with_gh)
                        else:
                            nc.tensor.matmul(pg[:], lhsT=wslice(wch, i, g),
                                             rhs=s[:, sl], start=is_first,
                                             stop=not with_gh)
                        if with_gh:
                            nc.tensor.matmul(pg[:], lhsT=wslice(whh, i, g),
                                             rhs=hT[b][:, sl], start=False,
                                             stop=True)

                    gx_mm(pr, 0, True, True)
                    nc.tensor.matmul(phn[:], lhsT=wslice(whh, i, 2),
                                     rhs=hT[b][:, sl], start=True, stop=True)
                    gx_mm(pz, 1, True, True)
                    gx_mm(pxn, 2, True, False)
                    banks.append((pr, phn, pz, pxn))

                # DVE: drain hn/xn psum banks immediately (adds bias, narrows
                # to fp16) so bank recycling never gates the PE.
                hn_ = [None] * 2
                xn_ = [None] * 2
                for nh in range(2):
                    hn_[nh] = ew_pool.tile([P, HALF], F16, tag="ew", name=f"hn{nh}")
                    nc.vector.tensor_scalar_add(hn_[nh][:], banks[nh][1][:],
                                                bslice(i, 3))
                    xn_[nh] = ew_pool.tile([P, HALF], F16, tag="ew", name=f"xn{nh}")
                    nc.vector.tensor_scalar_add(xn_[nh][:], banks[nh][3][:],
                                                bslice(i, 2))
                # previous batch's deferred final-add now that the drains are
                # ahead of it in the DVE queue
                if nh_prev is not None:
                    emit_nh(i, *nh_prev)
                # ACT: both sigmoids for both halves (inputs ready early)
                r = [None] * 2
                zc = [None] * 2
                for nh in range(2):
                    r[nh] = ew_pool.tile([P, HALF], F16, tag="ew", name=f"r{nh}")
                    nc.scalar.activation(r[nh][:], banks[nh][0][:], AF.Sigmoid,
                                         bias=bslice(i, 0))
                    zc[nh] = ew_pool.tile([P, HALF], F16, tag="ew", name=f"zc{nh}")
                    nc.scalar.activation(zc[nh][:], banks[nh][2][:], AF.Sigmoid,
                                         bias=bslice(i, 1), scale=-1.0)
                # next batch's s copies land on ACT before the late tanh
                if s_copies_next is not None:
                    emit_s_copies(s_copies_next)
                # DVE: t/u chains
                u = [None] * 2
                for nh in range(2):
                    t = ew_pool.tile([P, HALF], F16, tag="ew")
                    nc.vector.tensor_mul(t[:], hn_[nh][:], r[nh][:])
                    u[nh] = ew_pool.tile([P, HALF], F16, tag="ew", name=f"u{nh}")
                    nc.vector.tensor_add(u[nh][:], xn_[nh][:], t[:])
                # ACT: tanh; Pool: d, e; DVE: new_h
                nt = [None] * 2
                for nh in range(2):
                    nt[nh] = ew_pool.tile([P, HALF], F16, tag="ew", name=f"nt{nh}")
                    nc.scalar.activation(nt[nh][:], u[nh][:], AF.Tanh)
                e = [None] * 2
                for nh in range(2):
                    sl = slice(nh * HALF, (nh + 1) * HALF)
                    d = ew_pool.tile([P, HALF], F16, tag="ew")
                    nc.gpsimd.tensor_sub(d[:], nt[nh][:], hT[b][:, sl])
                    e[nh] = ew_pool.tile([P, HALF], F16, tag="ew", name=f"e{nh}")
                    nc.gpsimd.tensor_mul(e[nh][:], zc[nh][:], d[:])

                old_hT = hT[b]
                hT[b] = new_h
                return (b, old_hT, new_h, e)

            for i in range(L):
                s_q = [emit_s_mm(i, 0)]
                emit_s_copies(s_q[0])
                s_q.append(emit_s_mm(i, 1))
                nh_prev = None
                for b in range(BPC):
                    # NOTE: copies(b+1) (inside emit_gates) MUST be emitted
                    # before s-matmuls(b+2) reuse their ps_s banks -- the
                    # pool's WAR tracking only sees already-emitted readers.
                    cur = s_q.pop(0)
                    nh_prev = emit_gates(i, b, cur, s_q[0] if s_q else None,
                                         nh_prev)
                    if b + 2 < BPC:
                        s_q.append(emit_s_mm(i, b + 2))
                emit_nh(i, *nh_prev)

    nc.finalize()
    return nc


def _prep_weights(weight, w_ih, w_hh, b_ih, b_hh):
    weight = np.asarray(weight, np.float32)
    w_ih = np.asarray(w_ih, np.float32)
    w_hh = np.asarray(w_hh, np.float32)
    b_ih = np.asarray(b_ih, np.float32)
    b_hh = np.asarray(b_hh, np.float32)

    # fused input-gate weight: gx = s @ (W @ w_ih^T), as [C, L, 3, C];
    # kept in fp32 (the PE consumes it as fp32r -- same speed as fp16 at
    # 512-wide moving tiles, and the gate-weight rounding error would
    # otherwise amplify ~4x per layer through A^T)
    wc = np.einsum("lcd,lgd->lcg", weight, w_ih)          # [L, C, 3C]
    wch = wc.astype(np.float32)
    wch16 = wc.astype(np.float16)
    wcl16 = (wc - wch16.astype(np.float32)).astype(np.float16)
    whh_t = np.transpose(w_hh, (0, 2, 1)).astype(np.float16)  # [L, C, 3C]

    def to_clgd(a):  # [l, C, 3C] -> [C, l, 3, C]
        nl = a.shape[0]
        return np.ascontiguousarray(
            np.transpose(a.reshape(nl, C, 3, C), (1, 0, 2, 3)))

    bias = np.empty((C, L, 4), np.float32)
    bias[:, :, 0] = (b_ih[:, 0:C] + b_hh[:, 0:C]).T
    bias[:, :, 1] = -(b_ih[:, C:2 * C] + b_hh[:, C:2 * C]).T
    bias[:, :, 2] = b_ih[:, 2 * C:3 * C].T
    bias[:, :, 3] = b_hh[:, 2 * C:3 * C].T

    return (to_clgd(wch), to_clgd(wch16[:2]), to_clgd(wcl16[:2]),
            to_clgd(whh_t), bias)


def kernel(x, adj, mask, weight, w_ih, w_hh, b_ih, b_hh, _run_kwargs=None):
    # fp16 on device: exact for the binary adjacency; x/weights rounded
    x32 = np.asarray(x, np.float32)
    x = x32.astype(np.float16)
    # layer-0 s-matmul consumes x at ~fp32 via a hi/lo fp16 split: rounding
    # x before A^T x injects noise aligned with A's top singular directions,
    # which compounds ~4x per layer (2.7e-2 final vs 2e-3 with the split)
    xlo = (x32 - x.astype(np.float32)).astype(np.float16)
    xT = np.ascontiguousarray(np.transpose(x, (0, 2, 1)))
    adj = np.asarray(adj, np.float32).astype(np.float16)
    mask = np.asarray(mask, np.float32)
    wch, wch16, wcl16, whh, bias = _prep_weights(weight, w_ih, w_hh,
                                                 b_ih, b_hh)

    if "nc" not in _PROGRAM_CACHE:
        _PROGRAM_CACHE["nc"] = _build_program()
    nc = _PROGRAM_CACHE["nc"]

    in_maps = []
    for c in range(NCORES):
        sl = slice(c * BPC, (c + 1) * BPC)
        in_maps.append({
            "x": np.ascontiguousarray(x[sl]),
            "xlo": np.ascontiguousarray(xlo[sl]),
            "xT": xT[sl],
            "adj": np.ascontiguousarray(adj[sl]),
            "wch": wch, "wch16": wch16, "wcl16": wcl16,
            "whh": whh, "bias": bias,
        })

    res = run_bass_kernel_spmd(nc, in_maps, list(range(NCORES)),
                               **(_run_kwargs or {}))
    # y comes back feature-major fp16 [BPC, C, N]; widen + un-transpose
    y = np.concatenate([r["y"] for r in res.results], axis=0)
    y = np.transpose(y.astype(np.float32), (0, 2, 1)) * mask[:, :, None]
    if _run_kwargs:
        kernel.last_results = res
    return y
